# revision 11
# baseline (speedup 1.0000x reference)
"""AnomalyTransformer forward on 8 TRN2 NeuronCores.

Sharding: pure data-parallel over batch B=8 -> one batch element per core,
zero collectives. Each core runs the full 3-layer model on its [512, 55]
sequence.

Device design notes:
  - Activations kept TRANSPOSED in SBUF (hT[d, i]) so every projection uses
    natural-layout weights as lhsT and hT as the moving rhs.
  - All heavy matmuls in bf16 (inputs pre-cast on host), fp32 PSUM accum.
  - sigma path: sig = h @ (Wq@Ws) folded on host, computed in fp32.
  - Softmax without max-subtraction (logits are O(1e-1) by construction);
    exp() emits the unnormalized scores, accum_out gives the row-sum for
    free, normalization is one per-partition tensor_scalar multiply.
  - Attention needs scores in both [i,j] (softmax rows / series output) and
    [j,i] (lhsT for the V matmul) orientations: both are computed directly
    on the TensorEngine (cheaper than transposing 512x512 per head).
  - prior = exp(d2 * s) * c with s = -1/(2 sigma^2), c = 1/(sqrt(2pi) sigma)
    per-partition scalars; d2 = dist^2 precomputed on host.
  - Only ACT table set used is exp_and_others (exp/tanh/identity/copy)
    plus gelu for FFN; sigmoid is computed via exp + DVE reciprocal.
  - series/prior/sigma outputs are written as bf16 (fp32 exponent range,
    0.4% mantissa rounding, way under the tolerance) halving the dominant
    75MB/core output-DMA cost; host upcasts to fp32.
"""

import math
import os
import sys

import numpy as np

for _p in ("/root/.axon_site/_ro/trn_rl_repo", "/opt/trn_rl_repo"):
    if os.path.isdir(_p) and _p not in sys.path:
        sys.path.append(_p)

import ml_dtypes

BF16 = ml_dtypes.bfloat16

B, L, C_IN, D, H, NL, DFF, C_OUT = 8, 512, 55, 512, 8, 3, 2048, 55
E = D // H          # 64 head dim
P = 128
NB = D // P         # 4 blocks of 128 over d_model / tokens
FB = DFF // P       # 16 blocks over d_ff
SCALE = 1.0 / math.sqrt(E)
LN3 = math.log(3.0)
INV_SQRT_2PI = 1.0 / math.sqrt(2.0 * math.pi)

_GRAPH_CACHE = {}


def build_graph():
    import concourse.bass as bass
    import concourse.mybir as mybir
    import concourse.tile as tile
    from concourse import bacc
    from concourse.masks import make_identity

    f32 = mybir.dt.float32
    bf16 = mybir.dt.bfloat16
    AF = mybir.ActivationFunctionType
    OP = mybir.AluOpType

    nc = bacc.Bacc("TRN2", target_bir_lowering=False, debug=False)

    # ---------------- DRAM I/O (per-core shard, host-side layouts) --------
    xwin0 = nc.dram_tensor("xwin0", [P, L], bf16, kind="ExternalInput")
    xwin1 = nc.dram_tensor("xwin1", [3 * C_IN - P, L], bf16, kind="ExternalInput")
    wconv0 = nc.dram_tensor("wconv0", [P, D], bf16, kind="ExternalInput")
    wconv1 = nc.dram_tensor("wconv1", [3 * C_IN - P, D], bf16, kind="ExternalInput")
    posT = nc.dram_tensor("posT", [P, NB, L], f32, kind="ExternalInput")
    d2 = nc.dram_tensor("d2", [P, NB, L], f32, kind="ExternalInput")

    wq, wk, wv, wo, w1, w2, wqs = [], [], [], [], [], [], []
    bq, bk, b1r = [], [], []
    bvr, bor, b2r, bqsr = [], [], [], []
    for l in range(NL):
        wq.append(nc.dram_tensor(f"wq{l}", [P, NB, D], bf16, kind="ExternalInput"))
        wk.append(nc.dram_tensor(f"wk{l}", [P, NB, D], bf16, kind="ExternalInput"))
        wv.append(nc.dram_tensor(f"wv{l}", [P, NB, D], bf16, kind="ExternalInput"))
        wo.append(nc.dram_tensor(f"wo{l}", [P, NB, D], bf16, kind="ExternalInput"))
        w1.append(nc.dram_tensor(f"w1{l}", [P, NB, DFF], bf16, kind="ExternalInput"))
        w2.append(nc.dram_tensor(f"w2{l}", [P, FB, D], bf16, kind="ExternalInput"))
        wqs.append(nc.dram_tensor(f"wqs{l}", [P, NB, H], f32, kind="ExternalInput"))
        bq.append(nc.dram_tensor(f"bq{l}", [P, NB], f32, kind="ExternalInput"))
        bk.append(nc.dram_tensor(f"bk{l}", [P, NB], f32, kind="ExternalInput"))
        b1r.append(nc.dram_tensor(f"b1{l}", [P, FB], f32, kind="ExternalInput"))
        bvr.append(nc.dram_tensor(f"bv{l}", [1, D], f32, kind="ExternalInput"))
        bor.append(nc.dram_tensor(f"bo{l}", [1, D], f32, kind="ExternalInput"))
        b2r.append(nc.dram_tensor(f"b2{l}", [1, D], f32, kind="ExternalInput"))
        bqsr.append(nc.dram_tensor(f"bqs{l}", [1, H], f32, kind="ExternalInput"))
    wp = nc.dram_tensor("wp", [P, NB, C_OUT], bf16, kind="ExternalInput")
    bpr = nc.dram_tensor("bp", [1, C_OUT], f32, kind="ExternalInput")

    out_d = nc.dram_tensor("out", [P, NB, C_OUT], f32, kind="ExternalOutput")
    ser_d = nc.dram_tensor("series", [NL, H, P, NB, L], bf16, kind="ExternalOutput")
    pri_d = nc.dram_tensor("prior", [NL, H, P, NB, L], bf16, kind="ExternalOutput")
    sig_d = nc.dram_tensor("sigma", [NL, H, P, NB, L], bf16, kind="ExternalOutput")

    from contextlib import ExitStack

    with tile.TileContext(nc) as tc, ExitStack() as es:
        const = es.enter_context(tc.tile_pool(name="const", bufs=1))
        persist = es.enter_context(tc.tile_pool(name="persist", bufs=1))
        wpool = es.enter_context(tc.tile_pool(name="weights", bufs=1))
        act = es.enter_context(tc.tile_pool(name="acts", bufs=2))
        head = es.enter_context(tc.tile_pool(name="head", bufs=2))
        small = es.enter_context(tc.tile_pool(name="small", bufs=2))
        pmain = es.enter_context(tc.tile_pool(name="pmain", bufs=3, space="PSUM"))
        ptrans = es.enter_context(tc.tile_pool(name="ptrans", bufs=1, space="PSUM"))
        psmall = es.enter_context(tc.tile_pool(name="psmall", bufs=2, space="PSUM"))
        prow = es.enter_context(tc.tile_pool(name="prow", bufs=1, space="PSUM"))

        ident = const.tile([P, P], bf16)
        make_identity(nc, ident)
        ident_f = const.tile([P, P], f32)
        make_identity(nc, ident_f)
        ones = const.tile([1, L], f32)
        nc.vector.memset(ones, 1.0)
        ones_col_bf = const.tile([P, 1], bf16)
        nc.vector.memset(ones_col_bf, 1.0)
        bias_u = const.tile([P, 1], f32)       # ln3 * 1e-5 (sigmoid +1e-5 shift)
        nc.vector.memset(bias_u, LN3 * 1e-5)

        # persistent SBUF
        hT = persist.tile([P, NB, L], f32)        # transposed activations
        hTb = persist.tile([P, NB, L], bf16)      # bf16 shadow for matmuls
        d2_sb = persist.tile([P, NB, L], f32)
        nc.sync.dma_start(out=d2_sb[:], in_=d2.ap())

        # ---------------- embedding: circular conv (as matmul) + pos -----
        pos_sb = persist.tile([P, NB, L], f32)
        nc.sync.dma_start(out=pos_sb[:], in_=posT.ap())
        xw0 = persist.tile([P, L], bf16)
        xw1 = persist.tile([3 * C_IN - P, L], bf16)
        nc.sync.dma_start(out=xw0[:], in_=xwin0.ap())
        nc.sync.dma_start(out=xw1[:], in_=xwin1.ap())
        wc0 = persist.tile([P, D], bf16)
        wc1 = persist.tile([3 * C_IN - P, D], bf16)
        nc.sync.dma_start(out=wc0[:], in_=wconv0.ap())
        nc.sync.dma_start(out=wc1[:], in_=wconv1.ap())

        for db in range(NB):
            ps = pmain.tile([P, L], f32, tag="ps_main")
            nc.tensor.matmul(ps, wc0[:, db * P:(db + 1) * P], xw0[:], start=True, stop=False)
            nc.tensor.matmul(ps, wc1[:, db * P:(db + 1) * P], xw1[:], start=False, stop=True)
            nc.vector.tensor_add(out=hT[:, db, :], in0=ps, in1=pos_sb[:, db, :])
            nc.vector.tensor_copy(out=hTb[:, db, :], in_=hT[:, db, :])

        # ---------------- layers -----------------------------------------
        for l in range(NL):
            # weight loads (overlap with compute; slots free as layer ends)
            wq_s = wpool.tile([P, NB, D], bf16, tag="wq")
            wk_s = wpool.tile([P, NB, D], bf16, tag="wk")
            wv_s = wpool.tile([P, NB, D], bf16, tag="wv")
            wo_s = wpool.tile([P, NB, D], bf16, tag="wo")
            w1_s = wpool.tile([P, NB, DFF], bf16, tag="w1")
            w2_s = wpool.tile([P, FB, D], bf16, tag="w2")
            wqs_s = wpool.tile([P, NB, H], f32, tag="wqs")
            bq_s = wpool.tile([P, NB], f32, tag="bq")
            bk_s = wpool.tile([P, NB], f32, tag="bk")
            b1_s = wpool.tile([P, FB], f32, tag="b1")
            bv_s = wpool.tile([1, D], f32, tag="bv")
            bo_s = wpool.tile([1, D], f32, tag="bo")
            b2_s = wpool.tile([1, D], f32, tag="b2")
            bqs_s = wpool.tile([1, H], f32, tag="bqs")
            for t, dsrc in ((wq_s, wq[l]), (wk_s, wk[l]), (wv_s, wv[l]),
                            (wo_s, wo[l]), (w1_s, w1[l]), (w2_s, w2[l]),
                            (wqs_s, wqs[l]), (bq_s, bq[l]), (bk_s, bk[l]),
                            (b1_s, b1r[l]), (bv_s, bvr[l]), (bo_s, bor[l]),
                            (b2_s, b2r[l]), (bqs_s, bqsr[l])):
                nc.sync.dma_start(out=t[:], in_=dsrc.ap())

            # q/k in transposed layout [d_out, i], bf16; bias per-partition
            qT = act.tile([P, NB, L], bf16, tag="qT")
            kT = act.tile([P, NB, L], bf16, tag="kT")
            for (dst, w_s, b_s) in ((qT, wq_s, bq_s), (kT, wk_s, bk_s)):
                for db in range(NB):
                    ps = pmain.tile([P, L], f32, tag="ps_main")
                    for ch in range(NB):
                        nc.tensor.matmul(ps, w_s[:, ch, db * P:(db + 1) * P],
                                         hTb[:, ch, :], start=(ch == 0), stop=(ch == NB - 1))
                    nc.vector.tensor_scalar_add(dst[:, db, :], ps, b_s[:, db:db + 1])

            # v in natural layout [i, d], bias via rank-1 ones matmul
            v_s = act.tile([P, NB, D], bf16, tag="v")
            for ib in range(NB):
                ps = pmain.tile([P, D], f32, tag="ps_main")
                for ch in range(NB):
                    nc.tensor.matmul(ps, hTb[:, ch, ib * P:(ib + 1) * P],
                                     wv_s[:, ch, :], start=(ch == 0), stop=False)
                nc.tensor.matmul(ps, ones[0:1, 0:P], bv_s[:], start=False, stop=True)
                nc.vector.tensor_copy(out=v_s[:, ib, :], in_=ps)

            # sigma projection in fp32: sig = h @ (Wq Ws) + (bq Ws + bs)
            sig_s = small.tile([P, NB, H], f32, tag="sig")
            for ib in range(NB):
                ps = psmall.tile([P, H], f32, tag="ps_small")
                for ch in range(NB):
                    nc.tensor.matmul(ps, hT[:, ch, ib * P:(ib + 1) * P],
                                     wqs_s[:, ch, :], start=(ch == 0), stop=False)
                nc.tensor.matmul(ps, ones[0:1, 0:P], bqs_s[:], start=False, stop=True)
                nc.vector.tensor_copy(out=sig_s[:, ib, :], in_=ps)

            # sigma transform chain (fp32, exp/ln only):
            #   sgm = 1/(1+exp(-5 sig)); sigma = 3^(sgm+1e-5) - 1
            #   s = -1/(2 sigma^2); lnc = -ln(sqrt(2pi) sigma)
            t_e = small.tile([P, NB, H], f32, tag="sg_t1")
            nc.scalar.activation(t_e[:], sig_s[:], AF.Exp, scale=-5.0)
            nc.vector.tensor_scalar_add(t_e[:], t_e[:], 1.0)
            sgm = small.tile([P, NB, H], f32, tag="sg_t2")
            nc.vector.reciprocal(sgm[:], t_e[:])
            sigma_f = small.tile([P, NB, H], f32, tag="sigma")
            nc.scalar.activation(sigma_f[:], sgm[:], AF.Exp, scale=LN3, bias=bias_u[:])
            nc.vector.tensor_scalar_add(sigma_f[:], sigma_f[:], -1.0)
            sigma_b = small.tile([P, NB, H], bf16, tag="sigma_b")
            nc.vector.tensor_copy(out=sigma_b[:], in_=sigma_f[:])
            sq = small.tile([P, NB, H], f32, tag="sg_t3")
            nc.vector.tensor_mul(out=sq[:], in0=sigma_f[:], in1=sigma_f[:])
            s_sb = small.tile([P, NB, H], f32, tag="s_exp")
            nc.vector.reciprocal(s_sb[:], sq[:])
            nc.vector.tensor_scalar_mul(s_sb[:], s_sb[:], -0.5)
            lnc = small.tile([P, NB, H], f32, tag="lnc")
            nc.scalar.activation(lnc[:], sigma_f[:], AF.Ln, scale=math.sqrt(2 * math.pi))
            nc.vector.tensor_scalar_mul(lnc[:], lnc[:], -1.0)

            attn = act.tile([P, NB, D], bf16, tag="attn")

            for h in range(H):
                hp0 = E * (h % 2)
                hb = h // 2

                # scores [j, i] -> unnormalized exp (lhsT for the V matmul)
                expST = head.tile([P, NB, L], bf16, tag="expST")
                for jb in range(NB):
                    ps = pmain.tile([P, L], f32, tag="ps_main")
                    nc.tensor.matmul(ps, kT[hp0:hp0 + E, hb, jb * P:(jb + 1) * P],
                                     qT[hp0:hp0 + E, hb, :], start=True, stop=True)
                    nc.scalar.activation(expST[:, jb, :], ps, AF.Exp, scale=SCALE)

                # rowsum over j via ones-matmul (partition reduce on PE)
                psr = prow.tile([1, L], f32, tag="ps_row")
                for jb in range(NB):
                    nc.tensor.matmul(psr, ones_col_bf[:], expST[:, jb, :],
                                     start=(jb == 0), stop=(jb == NB - 1))
                rs_row = head.tile([1, L], f32, tag="rs_row")
                nc.vector.reciprocal(rs_row[:], psr)
                # transpose 1/Z back to per-partition columns [P, NB]
                rz_col = head.tile([P, NB], f32, tag="rz_col")
                for ib in range(NB):
                    pst1 = ptrans.tile([P, 1], f32, tag="ps_tr1")
                    nc.tensor.transpose(pst1, rs_row[0:1, ib * P:(ib + 1) * P],
                                        ident_f[0:1, 0:1])
                    nc.vector.tensor_copy(out=rz_col[:, ib:ib + 1], in_=pst1)
                nlz_col = head.tile([P, NB], f32, tag="nlz_col")
                nc.scalar.activation(nlz_col[:], rz_col[:], AF.Ln)

                # scores [i, j]: series = exp(s/8 - lnZ) directly normalized
                ser = head.tile([P, NB, L], bf16, tag="series")
                for ib in range(NB):
                    ps = pmain.tile([P, L], f32, tag="ps_main")
                    nc.tensor.matmul(ps, qT[hp0:hp0 + E, hb, ib * P:(ib + 1) * P],
                                     kT[hp0:hp0 + E, hb, :], start=True, stop=True)
                    nc.scalar.activation(ser[:, ib, :], ps, AF.Exp, scale=SCALE,
                                         bias=nlz_col[:, ib:ib + 1])
                nc.sync.dma_start(out=ser_d.ap()[l, h], in_=ser[:])

                # attn_out[i, e_head] = (expST.T @ v_head) * (1/Z)
                for ib in range(NB):
                    psA = psmall.tile([P, E], f32, tag="ps_small")
                    for jb in range(NB):
                        nc.tensor.matmul(psA, expST[:, jb, ib * P:(ib + 1) * P],
                                         v_s[:, jb, h * E:(h + 1) * E],
                                         start=(jb == 0), stop=(jb == NB - 1))
                    nc.vector.tensor_scalar_mul(attn[:, ib, h * E:(h + 1) * E], psA,
                                                rz_col[:, ib:ib + 1])

                # prior = exp(d2 * s + lnc)   (c folded into the bias)
                pri = head.tile([P, NB, L], bf16, tag="prior")
                for ib in range(NB):
                    nc.scalar.activation(pri[:, ib, :], d2_sb[:, ib, :], AF.Exp,
                                         scale=s_sb[:, ib, h:h + 1],
                                         bias=lnc[:, ib, h:h + 1])
                nc.sync.dma_start(out=pri_d.ap()[l, h], in_=pri[:])

                # sigma broadcast to [*, L] (DVE stride-0 read)
                sgb = head.tile([P, NB, L], bf16, tag="sigbc")
                for ib in range(NB):
                    nc.vector.tensor_copy(
                        out=sgb[:, ib, :],
                        in_=sigma_b[:, ib, h:h + 1].to_broadcast((P, L)))
                nc.sync.dma_start(out=sig_d.ap()[l, h], in_=sgb[:])

            # attn^T via PE transpose (bf16 128x128 tiles)
            attnT = act.tile([P, NB, L], bf16, tag="attnT")
            for db in range(NB):
                for ib in range(NB):
                    pst = ptrans.tile([P, P], bf16, tag="ps_tr")
                    nc.tensor.transpose(pst, attn[:, ib, db * P:(db + 1) * P], ident)
                    nc.vector.tensor_copy(out=attnT[:, db, ib * P:(ib + 1) * P], in_=pst)

            # h += attn @ Wo + bo   (in transposed layout)
            for db in range(NB):
                ps = pmain.tile([P, L], f32, tag="ps_main")
                for ch in range(NB):
                    nc.tensor.matmul(ps, wo_s[:, ch, db * P:(db + 1) * P],
                                     attnT[:, ch, :], start=(ch == 0), stop=False)
                nc.tensor.matmul(ps, bo_s[0:1, db * P:(db + 1) * P], ones[:],
                                 start=False, stop=True)
                nc.vector.tensor_add(out=hT[:, db, :], in0=hT[:, db, :], in1=ps)
                nc.vector.tensor_copy(out=hTb[:, db, :], in_=hT[:, db, :])

            # FFN
            y1 = act.tile([P, FB, L], bf16, tag="y1")
            for fb in range(FB):
                ps = pmain.tile([P, L], f32, tag="ps_main")
                for ch in range(NB):
                    nc.tensor.matmul(ps, w1_s[:, ch, fb * P:(fb + 1) * P],
                                     hTb[:, ch, :], start=(ch == 0), stop=(ch == NB - 1))
                nc.scalar.activation(y1[:, fb, :], ps, AF.Gelu, bias=b1_s[:, fb:fb + 1])
            for db in range(NB):
                ps = pmain.tile([P, L], f32, tag="ps_main")
                for fc in range(FB):
                    nc.tensor.matmul(ps, w2_s[:, fc, db * P:(db + 1) * P],
                                     y1[:, fc, :], start=(fc == 0), stop=False)
                nc.tensor.matmul(ps, b2_s[0:1, db * P:(db + 1) * P], ones[:],
                                 start=False, stop=True)
                nc.vector.tensor_add(out=hT[:, db, :], in0=hT[:, db, :], in1=ps)
                nc.vector.tensor_copy(out=hTb[:, db, :], in_=hT[:, db, :])

        # ---------------- output projection -------------------------------
        wp_s = persist.tile([P, NB, C_OUT], bf16)
        bp_s = persist.tile([1, C_OUT], f32)
        nc.sync.dma_start(out=wp_s[:], in_=wp.ap())
        nc.sync.dma_start(out=bp_s[:], in_=bpr.ap())
        out_sb = persist.tile([P, NB, C_OUT], f32)
        for ib in range(NB):
            ps = psmall.tile([P, C_OUT], f32, tag="ps_small")
            for ch in range(NB):
                nc.tensor.matmul(ps, hTb[:, ch, ib * P:(ib + 1) * P],
                                 wp_s[:, ch, :], start=(ch == 0), stop=False)
            nc.tensor.matmul(ps, ones[0:1, 0:P], bp_s[:], start=False, stop=True)
            nc.vector.tensor_copy(out=out_sb[:, ib, :], in_=ps)
        nc.sync.dma_start(out=out_d.ap(), in_=out_sb[:])

    nc.compile()
    return nc


def _prep_host_inputs(inputs):
    """Build the per-core in_maps (host-side layout transforms, bf16 casts)."""
    x = np.asarray(inputs["x"], np.float32)
    conv_w = np.asarray(inputs["conv_w"], np.float32)

    # positional embedding [L, D] -> transposed device layout [P, NB, L]
    pos_i = np.arange(L, dtype=np.float32)[:, None]
    div = np.exp(np.arange(0, D, 2, dtype=np.float32) * (-math.log(10000.0) / D))
    pe = np.zeros((L, D), np.float32)
    pe[:, 0::2] = np.sin(pos_i * div)
    pe[:, 1::2] = np.cos(pos_i * div)
    posT = np.ascontiguousarray(
        pe.T.reshape(NB, P, L).transpose(1, 0, 2))  # [P, NB, L]

    dist = np.abs(np.arange(L, dtype=np.float32)[:, None]
                  - np.arange(L, dtype=np.float32)[None, :])
    d2 = np.ascontiguousarray((dist ** 2).reshape(NB, P, L).transpose(1, 0, 2))

    wconv = np.transpose(conv_w, (2, 1, 0)).reshape(3 * C_IN, D)  # [(k c), o]

    def t_layout(w):  # [D, N] -> [P, D//P, N]
        return np.ascontiguousarray(
            w.reshape(w.shape[0] // P, P, w.shape[1]).transpose(1, 0, 2))

    def col_layout(b):  # [D] -> [P, D//P]
        return np.ascontiguousarray(b.reshape(b.shape[0] // P, P).T)

    shared = {
        "wconv0": wconv[:P].astype(BF16),
        "wconv1": wconv[P:].astype(BF16),
        "posT": posT,
        "d2": d2,
        "wp": t_layout(np.asarray(inputs["Wp"], np.float32)).astype(BF16),
        "bp": np.asarray(inputs["bp"], np.float32).reshape(1, C_OUT),
    }
    for l in range(NL):
        Wq = np.asarray(inputs["Wq"][l], np.float64)
        Ws = np.asarray(inputs["Ws"][l], np.float64)
        bq_ = np.asarray(inputs["bq"][l], np.float64)
        bs_ = np.asarray(inputs["bs"][l], np.float64)
        shared[f"wq{l}"] = t_layout(np.asarray(inputs["Wq"][l], np.float32)).astype(BF16)
        shared[f"wk{l}"] = t_layout(np.asarray(inputs["Wk"][l], np.float32)).astype(BF16)
        shared[f"wv{l}"] = t_layout(np.asarray(inputs["Wv"][l], np.float32)).astype(BF16)
        shared[f"wo{l}"] = t_layout(np.asarray(inputs["Wo"][l], np.float32)).astype(BF16)
        shared[f"w1{l}"] = t_layout(np.asarray(inputs["W1"][l], np.float32)).astype(BF16)
        shared[f"w2{l}"] = t_layout(np.asarray(inputs["W2"][l], np.float32)).astype(BF16)
        shared[f"wqs{l}"] = t_layout((Wq @ Ws).astype(np.float32))
        shared[f"bqs{l}"] = (bq_ @ Ws + bs_).astype(np.float32).reshape(1, H)
        shared[f"bq{l}"] = col_layout(np.asarray(inputs["bq"][l], np.float32))
        shared[f"bk{l}"] = col_layout(np.asarray(inputs["bk"][l], np.float32))
        shared[f"b1{l}"] = col_layout(np.asarray(inputs["b1"][l], np.float32))
        shared[f"bv{l}"] = np.asarray(inputs["bv"][l], np.float32).reshape(1, D)
        shared[f"bo{l}"] = np.asarray(inputs["bo"][l], np.float32).reshape(1, D)
        shared[f"b2{l}"] = np.asarray(inputs["b2"][l], np.float32).reshape(1, D)

    in_maps = []
    for b in range(B):
        xb = x[b]  # [L, C_IN]
        # xwin[(k c), i] = x[(i - 1 + k) % L, c]
        xwin = np.concatenate([np.roll(xb, 1 - k, axis=0).T for k in range(3)],
                              axis=0).astype(BF16)  # [165, L]
        m = dict(shared)
        m["xwin0"] = np.ascontiguousarray(xwin[:P])
        m["xwin1"] = np.ascontiguousarray(xwin[P:])
        in_maps.append(m)
    return in_maps


def run_cores(in_maps, trace=False, trace_kwargs=None):
    from concourse.bass_utils import run_bass_kernel_spmd

    if "nc" not in _GRAPH_CACHE:
        _GRAPH_CACHE["nc"] = build_graph()
    nc = _GRAPH_CACHE["nc"]
    return run_bass_kernel_spmd(nc, in_maps, core_ids=list(range(B)),
                                trace=trace, **(trace_kwargs or {}))


def _assemble(results):
    outs, sers, pris, sigs = [], [], [], []
    for b in range(B):
        r = results[b]
        o = np.asarray(r["out"], np.float32)          # [P, NB, C_OUT]
        outs.append(o.transpose(1, 0, 2).reshape(L, C_OUT))

        def big(name):
            a = np.asarray(r[name], np.float32)       # [NL, H, P, NB, L]
            return a.transpose(0, 1, 3, 2, 4).reshape(NL, H, L, L)

        sers.append(big("series"))
        pris.append(big("prior"))
        sigs.append(big("sigma"))
    return (np.stack(outs).astype(np.float32),
            np.stack(sers, axis=1).astype(np.float32),
            np.stack(pris, axis=1).astype(np.float32),
            np.stack(sigs, axis=1).astype(np.float32))


def kernel(**inputs):
    in_maps = _prep_host_inputs(inputs)
    res = run_cores(in_maps)
    return _assemble(res.results)


# revision 17
# speedup vs baseline: 1.3766x; 1.3766x over previous
"""AnomalyTransformer forward on 8 TRN2 NeuronCores.

Sharding: pure data-parallel over batch B=8 -> one batch element per core,
zero collectives. Each core runs the full 3-layer model on its [512, 55]
sequence.

Device design notes:
  - Activations kept TRANSPOSED in SBUF (hT[d, i]) so every projection uses
    natural-layout weights as lhsT and hT as the moving rhs.
  - All heavy matmuls in bf16 (inputs pre-cast on host), fp32 PSUM accum.
  - sigma path: sig = h @ (Wq@Ws) folded on host, computed in fp32.
  - Softmax without max-subtraction (logits are O(1e-1) by construction);
    exp() emits the unnormalized scores, accum_out gives the row-sum for
    free, normalization is one per-partition tensor_scalar multiply.
  - Attention needs scores in both [i,j] (softmax rows / series output) and
    [j,i] (lhsT for the V matmul) orientations: both are computed directly
    on the TensorEngine (cheaper than transposing 512x512 per head).
  - prior = exp(d2 * s) * c with s = -1/(2 sigma^2), c = 1/(sqrt(2pi) sigma)
    per-partition scalars; d2 = dist^2 precomputed on host.
  - Only ACT table set used is exp_and_others (exp/tanh/identity/copy)
    plus gelu for FFN; sigmoid is computed via exp + DVE reciprocal.
  - series/prior/sigma outputs are written as bf16 (fp32 exponent range,
    0.4% mantissa rounding, way under the tolerance) halving the dominant
    75MB/core output-DMA cost; host upcasts to fp32.
"""

import math
import os
import sys

import numpy as np

for _p in ("/root/.axon_site/_ro/trn_rl_repo", "/opt/trn_rl_repo"):
    if os.path.isdir(_p) and _p not in sys.path:
        sys.path.append(_p)

import ml_dtypes

BF16 = ml_dtypes.bfloat16

B, L, C_IN, D, H, NL, DFF, C_OUT = 8, 512, 55, 512, 8, 3, 2048, 55
E = D // H          # 64 head dim
P = 128
NB = D // P         # 4 blocks of 128 over d_model / tokens
FB = DFF // P       # 16 blocks over d_ff
SCALE = 1.0 / math.sqrt(E)
LN3 = math.log(3.0)
INV_SQRT_2PI = 1.0 / math.sqrt(2.0 * math.pi)

_GRAPH_CACHE = {}


def build_graph():
    import concourse.bass as bass
    import concourse.mybir as mybir
    import concourse.tile as tile
    from concourse import bacc
    from concourse.masks import make_identity

    f32 = mybir.dt.float32
    bf16 = mybir.dt.bfloat16
    AF = mybir.ActivationFunctionType
    OP = mybir.AluOpType

    nc = bacc.Bacc("TRN2", target_bir_lowering=False, debug=False)

    # ---------------- DRAM I/O (per-core shard, host-side layouts) --------
    xwin0 = nc.dram_tensor("xwin0", [P, L], bf16, kind="ExternalInput")
    xwin1 = nc.dram_tensor("xwin1", [3 * C_IN - P, L], bf16, kind="ExternalInput")
    wconv0 = nc.dram_tensor("wconv0", [P, D], bf16, kind="ExternalInput")
    wconv1 = nc.dram_tensor("wconv1", [3 * C_IN - P, D], bf16, kind="ExternalInput")
    posT = nc.dram_tensor("posT", [P, NB, L], f32, kind="ExternalInput")
    d2 = nc.dram_tensor("d2", [P, NB, L], f32, kind="ExternalInput")

    wq, wk, wv, wo, w1, w2, wqs = [], [], [], [], [], [], []
    bq, bk, b1r = [], [], []
    bvr, bor, b2r, bqsr = [], [], [], []
    for l in range(NL):
        wq.append(nc.dram_tensor(f"wq{l}", [P, NB, D], bf16, kind="ExternalInput"))
        wk.append(nc.dram_tensor(f"wk{l}", [P, NB, D], bf16, kind="ExternalInput"))
        wv.append(nc.dram_tensor(f"wv{l}", [P, NB, D], bf16, kind="ExternalInput"))
        wo.append(nc.dram_tensor(f"wo{l}", [P, NB, D], bf16, kind="ExternalInput"))
        w1.append(nc.dram_tensor(f"w1{l}", [P, NB, DFF], bf16, kind="ExternalInput"))
        w2.append(nc.dram_tensor(f"w2{l}", [P, FB, D], bf16, kind="ExternalInput"))
        wqs.append(nc.dram_tensor(f"wqs{l}", [P, NB, H], f32, kind="ExternalInput"))
        bq.append(nc.dram_tensor(f"bq{l}", [P, NB], f32, kind="ExternalInput"))
        bk.append(nc.dram_tensor(f"bk{l}", [P, NB], f32, kind="ExternalInput"))
        b1r.append(nc.dram_tensor(f"b1{l}", [P, FB], f32, kind="ExternalInput"))
        bvr.append(nc.dram_tensor(f"bv{l}", [1, D], f32, kind="ExternalInput"))
        bor.append(nc.dram_tensor(f"bo{l}", [1, D], f32, kind="ExternalInput"))
        b2r.append(nc.dram_tensor(f"b2{l}", [1, D], f32, kind="ExternalInput"))
        bqsr.append(nc.dram_tensor(f"bqs{l}", [1, H], f32, kind="ExternalInput"))
    wp = nc.dram_tensor("wp", [P, NB, C_OUT], bf16, kind="ExternalInput")
    bpr = nc.dram_tensor("bp", [1, C_OUT], f32, kind="ExternalInput")

    out_d = nc.dram_tensor("out", [P, NB, C_OUT], f32, kind="ExternalOutput")
    ser_d = nc.dram_tensor("series", [NL, H, P, NB, L], bf16, kind="ExternalOutput")
    pri_d = nc.dram_tensor("prior", [NL, H, P, NB, L], bf16, kind="ExternalOutput")
    sig_d = nc.dram_tensor("sigma", [NL, H, P, NB, L], bf16, kind="ExternalOutput")

    from contextlib import ExitStack

    with tile.TileContext(nc) as tc, ExitStack() as es:
        const = es.enter_context(tc.tile_pool(name="const", bufs=1))
        persist = es.enter_context(tc.tile_pool(name="persist", bufs=1))
        wpool = es.enter_context(tc.tile_pool(name="weights", bufs=1))
        act = es.enter_context(tc.tile_pool(name="acts", bufs=2))
        head = es.enter_context(tc.tile_pool(name="head", bufs=2))
        small = es.enter_context(tc.tile_pool(name="small", bufs=2))
        pmain = es.enter_context(tc.tile_pool(name="pmain", bufs=4, space="PSUM"))
        ptrans = es.enter_context(tc.tile_pool(name="ptrans", bufs=2, space="PSUM"))
        psmall = es.enter_context(tc.tile_pool(name="psmall", bufs=2, space="PSUM"))

        ident = const.tile([P, P], bf16)
        make_identity(nc, ident)
        ones = const.tile([1, L], f32)
        nc.vector.memset(ones, 1.0)
        bias_u = const.tile([P, 1], f32)       # ln3 * 1e-5 (sigmoid +1e-5 shift)
        nc.vector.memset(bias_u, LN3 * 1e-5)

        # persistent SBUF
        hT = persist.tile([P, NB, L], f32)        # transposed activations
        hTb = persist.tile([P, NB, L], bf16)      # bf16 shadow for matmuls
        d2_sb = persist.tile([P, NB, L], f32)
        nc.sync.dma_start(out=d2_sb[:], in_=d2.ap())

        # ---------------- embedding: circular conv (as matmul) + pos -----
        pos_sb = persist.tile([P, NB, L], f32)
        nc.sync.dma_start(out=pos_sb[:], in_=posT.ap())
        xw0 = persist.tile([P, L], bf16)
        xw1 = persist.tile([3 * C_IN - P, L], bf16)
        nc.sync.dma_start(out=xw0[:], in_=xwin0.ap())
        nc.sync.dma_start(out=xw1[:], in_=xwin1.ap())
        wc0 = persist.tile([P, D], bf16)
        wc1 = persist.tile([3 * C_IN - P, D], bf16)
        nc.sync.dma_start(out=wc0[:], in_=wconv0.ap())
        nc.sync.dma_start(out=wc1[:], in_=wconv1.ap())

        for db in range(NB):
            ps = pmain.tile([P, L], f32, tag="ps_main")
            nc.tensor.matmul(ps, wc0[:, db * P:(db + 1) * P], xw0[:], start=True, stop=False)
            nc.tensor.matmul(ps, wc1[:, db * P:(db + 1) * P], xw1[:], start=False, stop=True)
            nc.vector.tensor_add(out=hT[:, db, :], in0=ps, in1=pos_sb[:, db, :])
            nc.vector.tensor_copy(out=hTb[:, db, :], in_=hT[:, db, :])

        # ---------------- layers -----------------------------------------
        for l in range(NL):
            # weight loads (overlap with compute; slots free as layer ends)
            wq_s = wpool.tile([P, NB, D], bf16, tag="wq")
            wk_s = wpool.tile([P, NB, D], bf16, tag="wk")
            wv_s = wpool.tile([P, NB, D], bf16, tag="wv")
            wo_s = wpool.tile([P, NB, D], bf16, tag="wo")
            w1_s = wpool.tile([P, NB, DFF], bf16, tag="w1")
            w2_s = wpool.tile([P, FB, D], bf16, tag="w2")
            wqs_s = wpool.tile([P, NB, H], f32, tag="wqs")
            bq_s = wpool.tile([P, NB], f32, tag="bq")
            bk_s = wpool.tile([P, NB], f32, tag="bk")
            b1_s = wpool.tile([P, FB], f32, tag="b1")
            bv_s = wpool.tile([1, D], f32, tag="bv")
            bo_s = wpool.tile([1, D], f32, tag="bo")
            b2_s = wpool.tile([1, D], f32, tag="b2")
            bqs_s = wpool.tile([1, H], f32, tag="bqs")
            for t, dsrc in ((wq_s, wq[l]), (wk_s, wk[l]), (wv_s, wv[l]),
                            (wo_s, wo[l]), (w1_s, w1[l]), (w2_s, w2[l]),
                            (wqs_s, wqs[l]), (bq_s, bq[l]), (bk_s, bk[l]),
                            (b1_s, b1r[l]), (bv_s, bvr[l]), (bo_s, bor[l]),
                            (b2_s, b2r[l]), (bqs_s, bqsr[l])):
                nc.sync.dma_start(out=t[:], in_=dsrc.ap())

            # q/k in transposed layout [d_out, i], bf16; bias per-partition
            qT = act.tile([P, NB, L], bf16, tag="qT")
            kT = act.tile([P, NB, L], bf16, tag="kT")
            for (dst, w_s, b_s) in ((qT, wq_s, bq_s), (kT, wk_s, bk_s)):
                for db in range(NB):
                    ps = pmain.tile([P, L], f32, tag="ps_main")
                    for ch in range(NB):
                        nc.tensor.matmul(ps, w_s[:, ch, db * P:(db + 1) * P],
                                         hTb[:, ch, :], start=(ch == 0), stop=(ch == NB - 1))
                    nc.vector.tensor_scalar_add(dst[:, db, :], ps, b_s[:, db:db + 1])

            # v in natural layout [i, d] extended with a ones column per head:
            # vx[:, ib, h*65 : h*65+64] = v_head, vx[:, ib, h*65+64] = 1.0
            # so the attention matmul's last output column is the softmax
            # denominator (free rowsum) for that i-block.
            v_s = act.tile([P, NB, H * (E + 1)], bf16, tag="v")
            nc.vector.memset(v_s[:, :, E::E + 1], 1.0)
            for ib in range(NB):
                ps = pmain.tile([P, D], f32, tag="ps_main")
                for ch in range(NB):
                    nc.tensor.matmul(ps, hTb[:, ch, ib * P:(ib + 1) * P],
                                     wv_s[:, ch, :], start=(ch == 0), stop=False)
                nc.tensor.matmul(ps, ones[0:1, 0:P], bv_s[:], start=False, stop=True)
                for h in range(H):
                    nc.vector.tensor_copy(out=v_s[:, ib, h * (E + 1):h * (E + 1) + E],
                                          in_=ps[:, h * E:(h + 1) * E])

            # sigma projection in fp32: sig = h @ (Wq Ws) + (bq Ws + bs)
            sig_s = small.tile([P, NB, H], f32, tag="sig")
            for ib in range(NB):
                ps = psmall.tile([P, H], f32, tag="ps_small")
                for ch in range(NB):
                    nc.tensor.matmul(ps, hT[:, ch, ib * P:(ib + 1) * P],
                                     wqs_s[:, ch, :], start=(ch == 0), stop=False)
                nc.tensor.matmul(ps, ones[0:1, 0:P], bqs_s[:], start=False, stop=True)
                nc.vector.tensor_copy(out=sig_s[:, ib, :], in_=ps)

            # sigma transform chain (fp32, exp/ln only):
            #   sgm = 1/(1+exp(-5 sig)); sigma = 3^(sgm+1e-5) - 1
            #   s = -1/(2 sigma^2); lnc = -ln(sqrt(2pi) sigma)
            t_e = small.tile([P, NB, H], f32, tag="sg_t1")
            nc.scalar.activation(t_e[:], sig_s[:], AF.Exp, scale=-5.0)
            nc.vector.tensor_scalar_add(t_e[:], t_e[:], 1.0)
            sgm = small.tile([P, NB, H], f32, tag="sg_t2")
            nc.vector.reciprocal(sgm[:], t_e[:])
            sigma_f = small.tile([P, NB, H], f32, tag="sigma")
            nc.scalar.activation(sigma_f[:], sgm[:], AF.Exp, scale=LN3, bias=bias_u[:])
            nc.vector.tensor_scalar_add(sigma_f[:], sigma_f[:], -1.0)
            sigma_b = small.tile([P, NB, H], bf16, tag="sigma_b")
            nc.vector.tensor_copy(out=sigma_b[:], in_=sigma_f[:])
            sq = small.tile([P, NB, H], f32, tag="sg_t3")
            nc.vector.tensor_mul(out=sq[:], in0=sigma_f[:], in1=sigma_f[:])
            s_sb = small.tile([P, NB, H], f32, tag="s_exp")
            nc.vector.reciprocal(s_sb[:], sq[:])
            nc.vector.tensor_scalar_mul(s_sb[:], s_sb[:], -0.5)
            lnc = small.tile([P, NB, H], f32, tag="lnc")
            nc.scalar.activation(lnc[:], sigma_f[:], AF.Ln, scale=math.sqrt(2 * math.pi))
            nc.vector.tensor_scalar_mul(lnc[:], lnc[:], -1.0)

            attn = act.tile([P, NB, D], bf16, tag="attn")

            for h in range(H):
                hp0 = E * (h % 2)
                hb = h // 2

                # scores [j, i] -> unnormalized exp (lhsT for the V matmul)
                expST = head.tile([P, NB, L], bf16, tag="expST")
                for jb in range(NB):
                    ps = pmain.tile([P, L], f32, tag="ps_main")
                    nc.tensor.matmul(ps, kT[hp0:hp0 + E, hb, jb * P:(jb + 1) * P],
                                     qT[hp0:hp0 + E, hb, :], start=True, stop=True)
                    nc.scalar.activation(expST[:, jb, :], ps, AF.Exp, scale=SCALE)

                # attn_unnorm | rowsum = expST.T @ [v_head | 1]
                rz_col = head.tile([P, NB], f32, tag="rz_col")
                for ib in range(NB):
                    psA = psmall.tile([P, E + 1], f32, tag="ps_small")
                    for jb in range(NB):
                        nc.tensor.matmul(psA, expST[:, jb, ib * P:(ib + 1) * P],
                                         v_s[:, jb, h * (E + 1):(h + 1) * (E + 1)],
                                         start=(jb == 0), stop=(jb == NB - 1))
                    nc.vector.reciprocal(rz_col[:, ib:ib + 1], psA[:, E:E + 1])
                    nc.vector.tensor_scalar_mul(attn[:, ib, h * E:(h + 1) * E],
                                                psA[:, 0:E], rz_col[:, ib:ib + 1])
                nlz_col = head.tile([P, NB], f32, tag="nlz_col")
                nc.scalar.activation(nlz_col[:], rz_col[:], AF.Ln)

                # scores [i, j]: series = exp(s/8 - lnZ) directly normalized
                ser = head.tile([P, NB, L], bf16, tag="series")
                for ib in range(NB):
                    ps = pmain.tile([P, L], f32, tag="ps_main")
                    nc.tensor.matmul(ps, qT[hp0:hp0 + E, hb, ib * P:(ib + 1) * P],
                                     kT[hp0:hp0 + E, hb, :], start=True, stop=True)
                    nc.scalar.activation(ser[:, ib, :], ps, AF.Exp, scale=SCALE,
                                         bias=nlz_col[:, ib:ib + 1])
                nc.sync.dma_start(out=ser_d.ap()[l, h], in_=ser[:])

                # prior = exp(d2 * s + lnc)   (c folded into the bias)
                pri = head.tile([P, NB, L], bf16, tag="prior")
                for ib in range(NB):
                    nc.scalar.activation(pri[:, ib, :], d2_sb[:, ib, :], AF.Exp,
                                         scale=s_sb[:, ib, h:h + 1],
                                         bias=lnc[:, ib, h:h + 1])
                nc.sync.dma_start(out=pri_d.ap()[l, h], in_=pri[:])

                # sigma broadcast to [*, L] (DVE stride-0 read)
                sgb = head.tile([P, NB, L], bf16, tag="sigbc")
                for ib in range(NB):
                    nc.vector.tensor_copy(
                        out=sgb[:, ib, :],
                        in_=sigma_b[:, ib, h:h + 1].to_broadcast((P, L)))
                nc.sync.dma_start(out=sig_d.ap()[l, h], in_=sgb[:])

            # attn^T via PE transpose (bf16 128x128 tiles)
            attnT = act.tile([P, NB, L], bf16, tag="attnT")
            for db in range(NB):
                for ib in range(NB):
                    pst = ptrans.tile([P, P], bf16, tag="ps_tr")
                    nc.tensor.transpose(pst, attn[:, ib, db * P:(db + 1) * P], ident)
                    nc.vector.tensor_copy(out=attnT[:, db, ib * P:(ib + 1) * P], in_=pst)

            # h += attn @ Wo + bo   (in transposed layout)
            for db in range(NB):
                ps = pmain.tile([P, L], f32, tag="ps_main")
                for ch in range(NB):
                    nc.tensor.matmul(ps, wo_s[:, ch, db * P:(db + 1) * P],
                                     attnT[:, ch, :], start=(ch == 0), stop=False)
                nc.tensor.matmul(ps, bo_s[0:1, db * P:(db + 1) * P], ones[:],
                                 start=False, stop=True)
                nc.vector.tensor_add(out=hT[:, db, :], in0=hT[:, db, :], in1=ps)
                nc.vector.tensor_copy(out=hTb[:, db, :], in_=hT[:, db, :])

            # FFN
            y1 = act.tile([P, FB, L], bf16, tag="y1")
            for fb in range(FB):
                ps = pmain.tile([P, L], f32, tag="ps_main")
                for ch in range(NB):
                    nc.tensor.matmul(ps, w1_s[:, ch, fb * P:(fb + 1) * P],
                                     hTb[:, ch, :], start=(ch == 0), stop=(ch == NB - 1))
                nc.scalar.activation(y1[:, fb, :], ps, AF.Gelu, bias=b1_s[:, fb:fb + 1])
            for db in range(NB):
                ps = pmain.tile([P, L], f32, tag="ps_main")
                for fc in range(FB):
                    nc.tensor.matmul(ps, w2_s[:, fc, db * P:(db + 1) * P],
                                     y1[:, fc, :], start=(fc == 0), stop=False)
                nc.tensor.matmul(ps, b2_s[0:1, db * P:(db + 1) * P], ones[:],
                                 start=False, stop=True)
                nc.vector.tensor_add(out=hT[:, db, :], in0=hT[:, db, :], in1=ps)
                nc.vector.tensor_copy(out=hTb[:, db, :], in_=hT[:, db, :])

        # ---------------- output projection -------------------------------
        wp_s = persist.tile([P, NB, C_OUT], bf16)
        bp_s = persist.tile([1, C_OUT], f32)
        nc.sync.dma_start(out=wp_s[:], in_=wp.ap())
        nc.sync.dma_start(out=bp_s[:], in_=bpr.ap())
        out_sb = persist.tile([P, NB, C_OUT], f32)
        for ib in range(NB):
            ps = psmall.tile([P, C_OUT], f32, tag="ps_small")
            for ch in range(NB):
                nc.tensor.matmul(ps, hTb[:, ch, ib * P:(ib + 1) * P],
                                 wp_s[:, ch, :], start=(ch == 0), stop=False)
            nc.tensor.matmul(ps, ones[0:1, 0:P], bp_s[:], start=False, stop=True)
            nc.vector.tensor_copy(out=out_sb[:, ib, :], in_=ps)
        nc.sync.dma_start(out=out_d.ap(), in_=out_sb[:])

    # The act-table chooser maps each ACT function to the first table set
    # containing it, which puts exp and ln in different sets -> a ~2.7us
    # table reload per head. Restrict exp/ln membership to the shared
    # natural_log_exp_and_others set (dict order preserved, so set ids stay
    # valid) so the whole kernel runs on one resident set + gelu.
    import concourse.bacc as bacc_mod
    from concourse.hw_specs import get_activation_tables as _real_gat

    def _patched_gat(arch):
        tables = _real_gat(arch)
        exp_t = mybir.ActivationFunctionType.Exp
        ln_t = mybir.ActivationFunctionType.Ln
        for name, fns in tables.items():
            if name != "natural_log_exp_and_others":
                fns.discard(exp_t)
                fns.discard(ln_t)
        return tables

    bacc_mod.get_activation_tables = _patched_gat
    try:
        nc.compile()
    finally:
        bacc_mod.get_activation_tables = _real_gat
    return nc


def _prep_host_inputs(inputs):
    """Build the per-core in_maps (host-side layout transforms, bf16 casts)."""
    x = np.asarray(inputs["x"], np.float32)
    conv_w = np.asarray(inputs["conv_w"], np.float32)

    # positional embedding [L, D] -> transposed device layout [P, NB, L]
    pos_i = np.arange(L, dtype=np.float32)[:, None]
    div = np.exp(np.arange(0, D, 2, dtype=np.float32) * (-math.log(10000.0) / D))
    pe = np.zeros((L, D), np.float32)
    pe[:, 0::2] = np.sin(pos_i * div)
    pe[:, 1::2] = np.cos(pos_i * div)
    posT = np.ascontiguousarray(
        pe.T.reshape(NB, P, L).transpose(1, 0, 2))  # [P, NB, L]

    dist = np.abs(np.arange(L, dtype=np.float32)[:, None]
                  - np.arange(L, dtype=np.float32)[None, :])
    d2 = np.ascontiguousarray((dist ** 2).reshape(NB, P, L).transpose(1, 0, 2))

    wconv = np.transpose(conv_w, (2, 1, 0)).reshape(3 * C_IN, D)  # [(k c), o]

    def t_layout(w):  # [D, N] -> [P, D//P, N]
        return np.ascontiguousarray(
            w.reshape(w.shape[0] // P, P, w.shape[1]).transpose(1, 0, 2))

    def col_layout(b):  # [D] -> [P, D//P]
        return np.ascontiguousarray(b.reshape(b.shape[0] // P, P).T)

    shared = {
        "wconv0": wconv[:P].astype(BF16),
        "wconv1": wconv[P:].astype(BF16),
        "posT": posT,
        "d2": d2,
        "wp": t_layout(np.asarray(inputs["Wp"], np.float32)).astype(BF16),
        "bp": np.asarray(inputs["bp"], np.float32).reshape(1, C_OUT),
    }
    for l in range(NL):
        Wq = np.asarray(inputs["Wq"][l], np.float64)
        Ws = np.asarray(inputs["Ws"][l], np.float64)
        bq_ = np.asarray(inputs["bq"][l], np.float64)
        bs_ = np.asarray(inputs["bs"][l], np.float64)
        shared[f"wq{l}"] = t_layout(np.asarray(inputs["Wq"][l], np.float32)).astype(BF16)
        shared[f"wk{l}"] = t_layout(np.asarray(inputs["Wk"][l], np.float32)).astype(BF16)
        shared[f"wv{l}"] = t_layout(np.asarray(inputs["Wv"][l], np.float32)).astype(BF16)
        shared[f"wo{l}"] = t_layout(np.asarray(inputs["Wo"][l], np.float32)).astype(BF16)
        shared[f"w1{l}"] = t_layout(np.asarray(inputs["W1"][l], np.float32)).astype(BF16)
        shared[f"w2{l}"] = t_layout(np.asarray(inputs["W2"][l], np.float32)).astype(BF16)
        shared[f"wqs{l}"] = t_layout((Wq @ Ws).astype(np.float32))
        shared[f"bqs{l}"] = (bq_ @ Ws + bs_).astype(np.float32).reshape(1, H)
        shared[f"bq{l}"] = col_layout(np.asarray(inputs["bq"][l], np.float32))
        shared[f"bk{l}"] = col_layout(np.asarray(inputs["bk"][l], np.float32))
        shared[f"b1{l}"] = col_layout(np.asarray(inputs["b1"][l], np.float32))
        shared[f"bv{l}"] = np.asarray(inputs["bv"][l], np.float32).reshape(1, D)
        shared[f"bo{l}"] = np.asarray(inputs["bo"][l], np.float32).reshape(1, D)
        shared[f"b2{l}"] = np.asarray(inputs["b2"][l], np.float32).reshape(1, D)

    in_maps = []
    for b in range(B):
        xb = x[b]  # [L, C_IN]
        # xwin[(k c), i] = x[(i - 1 + k) % L, c]
        xwin = np.concatenate([np.roll(xb, 1 - k, axis=0).T for k in range(3)],
                              axis=0).astype(BF16)  # [165, L]
        m = dict(shared)
        m["xwin0"] = np.ascontiguousarray(xwin[:P])
        m["xwin1"] = np.ascontiguousarray(xwin[P:])
        in_maps.append(m)
    return in_maps


def run_cores(in_maps, trace=False, trace_kwargs=None):
    from concourse.bass_utils import run_bass_kernel_spmd

    if "nc" not in _GRAPH_CACHE:
        _GRAPH_CACHE["nc"] = build_graph()
    nc = _GRAPH_CACHE["nc"]
    return run_bass_kernel_spmd(nc, in_maps, core_ids=list(range(B)),
                                trace=trace, **(trace_kwargs or {}))


def _assemble(results):
    outs, sers, pris, sigs = [], [], [], []
    for b in range(B):
        r = results[b]
        o = np.asarray(r["out"], np.float32)          # [P, NB, C_OUT]
        outs.append(o.transpose(1, 0, 2).reshape(L, C_OUT))

        def big(name):
            a = np.asarray(r[name], np.float32)       # [NL, H, P, NB, L]
            return a.transpose(0, 1, 3, 2, 4).reshape(NL, H, L, L)

        sers.append(big("series"))
        pris.append(big("prior"))
        sigs.append(big("sigma"))
    return (np.stack(outs).astype(np.float32),
            np.stack(sers, axis=1).astype(np.float32),
            np.stack(pris, axis=1).astype(np.float32),
            np.stack(sigs, axis=1).astype(np.float32))


def kernel(**inputs):
    in_maps = _prep_host_inputs(inputs)
    res = run_cores(in_maps)
    return _assemble(res.results)


# revision 21
# speedup vs baseline: 1.6604x; 1.2062x over previous
"""AnomalyTransformer forward on 8 TRN2 NeuronCores.

Sharding: pure data-parallel over batch B=8 -> one batch element per core,
zero collectives. Each core runs the full 3-layer model on its [512, 55]
sequence.

Device design notes:
  - Activations kept TRANSPOSED in SBUF (hT[d, i]) so every projection uses
    natural-layout weights as lhsT and hT as the moving rhs.
  - All heavy matmuls in bf16 (inputs pre-cast on host), fp32 PSUM accum.
  - sigma path: sig = h @ (Wq@Ws) folded on host, computed in fp32.
  - V is extended with a per-head ones column so the attention matmul's last
    output column is the softmax denominator (free per-partition rowsum);
    softmax normalization of the series output is folded into the exp bias
    (-ln Z per query row). No max-subtraction (logits are O(1e-1)).
  - prior = exp(d2 * s + ln c) with s = -1/(2 sigma^2), c = 1/(sqrt(2pi)
    sigma) as per-partition ACT scale/bias; computed only on a +-1-block
    band around the diagonal (it underflows to exact fp32 zero beyond
    |i-j| ~ 27), the rest is memset to zero on the otherwise-idle GpSimd.
  - ACT functions are restricted to one resident table set
    (natural_log_exp_and_others: exp/ln) plus gelu, to avoid ~2.7us
    ACT_TABLE_LOAD thrash; the per-set function map is pruned accordingly
    around compile.
  - Attention-phase ACT work that does not feed the residual stream
    (series/prior/sigma) is emitted after the FFN so it overlaps the
    PE-heavy FFN/QKV phases instead of starving the PE (HAM throttle).
  - series/prior/sigma outputs are written as bf16 (fp32 exponent range,
    0.4% mantissa rounding, far under tolerance), halving the dominant
    75MB/core output-DMA cost; host upcasts to fp32.
"""

import math
import os
import sys

import numpy as np

for _p in ("/root/.axon_site/_ro/trn_rl_repo", "/opt/trn_rl_repo"):
    if os.path.isdir(_p) and _p not in sys.path:
        sys.path.append(_p)

import ml_dtypes

BF16 = ml_dtypes.bfloat16

B, L, C_IN, D, H, NL, DFF, C_OUT = 8, 512, 55, 512, 8, 3, 2048, 55
E = D // H          # 64 head dim
P = 128
NB = D // P         # 4 blocks of 128 over d_model / tokens
FB = DFF // P       # 16 blocks over d_ff
HG = H // 2         # head pairs (row-group packed matmuls)
SCALE = 1.0 / math.sqrt(E)
LN3 = math.log(3.0)

_GRAPH_CACHE = {}


def build_graph():
    import concourse.bass as bass
    import concourse.mybir as mybir
    import concourse.tile as tile
    from concourse import bacc
    from concourse.masks import make_identity

    f32 = mybir.dt.float32
    bf16 = mybir.dt.bfloat16
    AF = mybir.ActivationFunctionType

    nc = bacc.Bacc("TRN2", target_bir_lowering=False, debug=False)

    # ---------------- DRAM I/O (per-core shard, host-side layouts) --------
    xwin0 = nc.dram_tensor("xwin0", [P, L], bf16, kind="ExternalInput")
    xwin1 = nc.dram_tensor("xwin1", [3 * C_IN - P, L], bf16, kind="ExternalInput")
    wconv0 = nc.dram_tensor("wconv0", [P, D], bf16, kind="ExternalInput")
    wconv1 = nc.dram_tensor("wconv1", [3 * C_IN - P, D], bf16, kind="ExternalInput")
    posT = nc.dram_tensor("posT", [P, NB, L], f32, kind="ExternalInput")
    d2 = nc.dram_tensor("d2", [P, NB, L], f32, kind="ExternalInput")

    wq, wk, wv, wo, w1, w2, wqs = [], [], [], [], [], [], []
    bq, bk, b1r = [], [], []
    bvr, bor, b2r, bqsr = [], [], [], []
    for l in range(NL):
        wq.append(nc.dram_tensor(f"wq{l}", [P, NB, D], bf16, kind="ExternalInput"))
        wk.append(nc.dram_tensor(f"wk{l}", [P, NB, D], bf16, kind="ExternalInput"))
        wv.append(nc.dram_tensor(f"wv{l}", [P, NB, D], bf16, kind="ExternalInput"))
        wo.append(nc.dram_tensor(f"wo{l}", [P, NB, D], bf16, kind="ExternalInput"))
        w1.append(nc.dram_tensor(f"w1{l}", [P, NB, DFF], bf16, kind="ExternalInput"))
        w2.append(nc.dram_tensor(f"w2{l}", [P, FB, D], bf16, kind="ExternalInput"))
        wqs.append(nc.dram_tensor(f"wqs{l}", [P, NB, H], f32, kind="ExternalInput"))
        bq.append(nc.dram_tensor(f"bq{l}", [P, NB], f32, kind="ExternalInput"))
        bk.append(nc.dram_tensor(f"bk{l}", [P, NB], f32, kind="ExternalInput"))
        b1r.append(nc.dram_tensor(f"b1{l}", [P, FB], f32, kind="ExternalInput"))
        bvr.append(nc.dram_tensor(f"bv{l}", [1, D], f32, kind="ExternalInput"))
        bor.append(nc.dram_tensor(f"bo{l}", [1, D], f32, kind="ExternalInput"))
        b2r.append(nc.dram_tensor(f"b2{l}", [1, D], f32, kind="ExternalInput"))
        bqsr.append(nc.dram_tensor(f"bqs{l}", [1, H], f32, kind="ExternalInput"))
    wp = nc.dram_tensor("wp", [P, NB, C_OUT], bf16, kind="ExternalInput")
    bpr = nc.dram_tensor("bp", [1, C_OUT], f32, kind="ExternalInput")

    out_d = nc.dram_tensor("out", [P, NB, C_OUT], f32, kind="ExternalOutput")
    ser_d = nc.dram_tensor("series", [NL, H, P, NB, L], bf16, kind="ExternalOutput")
    pri_d = nc.dram_tensor("prior", [NL, H, P, NB, L], bf16, kind="ExternalOutput")
    sig_d = nc.dram_tensor("sigma", [NL, H, P, NB, L], bf16, kind="ExternalOutput")

    from contextlib import ExitStack

    with tile.TileContext(nc) as tc, ExitStack() as es:
        const = es.enter_context(tc.tile_pool(name="const", bufs=1))
        persist = es.enter_context(tc.tile_pool(name="persist", bufs=1))
        wpool = es.enter_context(tc.tile_pool(name="weights", bufs=1))
        act = es.enter_context(tc.tile_pool(name="acts", bufs=2))
        head = es.enter_context(tc.tile_pool(name="head", bufs=2))
        small = es.enter_context(tc.tile_pool(name="small", bufs=2))
        pmain = es.enter_context(tc.tile_pool(name="pmain", bufs=4, space="PSUM"))
        ptrans = es.enter_context(tc.tile_pool(name="ptrans", bufs=2, space="PSUM"))
        psmall = es.enter_context(tc.tile_pool(name="psmall", bufs=2, space="PSUM"))

        ident = const.tile([P, P], bf16)
        make_identity(nc, ident)
        ones = const.tile([1, L], f32)
        nc.vector.memset(ones, 1.0)
        bias_u = const.tile([P, 1], f32)       # ln3 * 1e-5 (sigmoid +1e-5 shift)
        nc.vector.memset(bias_u, LN3 * 1e-5)

        # ---------------- embedding: circular conv (as matmul) + pos -----
        hT = persist.tile([P, NB, L], f32)        # transposed activations
        hTb = persist.tile([P, NB, L], bf16)      # bf16 shadow for matmuls

        with tc.tile_pool(name="embed", bufs=1) as epool:
            xw0 = epool.tile([P, L], bf16)
            xw1 = epool.tile([3 * C_IN - P, L], bf16)
            nc.sync.dma_start(out=xw0[:], in_=xwin0.ap())
            nc.sync.dma_start(out=xw1[:], in_=xwin1.ap())
            wc0 = epool.tile([P, D], bf16)
            wc1 = epool.tile([3 * C_IN - P, D], bf16)
            nc.sync.dma_start(out=wc0[:], in_=wconv0.ap())
            nc.sync.dma_start(out=wc1[:], in_=wconv1.ap())
            pos_sb = epool.tile([P, NB, L], f32)
            nc.sync.dma_start(out=pos_sb[:], in_=posT.ap())

            for db in range(NB):
                ps = pmain.tile([P, L], f32, tag="ps_main")
                nc.tensor.matmul(ps, wc0[:, db * P:(db + 1) * P], xw0[:], start=True, stop=False)
                nc.tensor.matmul(ps, wc1[:, db * P:(db + 1) * P], xw1[:], start=False, stop=True)
                nc.vector.tensor_add(out=hT[:, db, :], in0=ps, in1=pos_sb[:, db, :])
                nc.vector.tensor_copy(out=hTb[:, db, :], in_=hT[:, db, :])

        d2_sb = persist.tile([P, NB, L], f32)
        nc.sync.dma_start(out=d2_sb[:], in_=d2.ap())

        # ---------------- layers -----------------------------------------
        for l in range(NL):
            # weight loads (overlap with compute; slots free as layer ends)
            wq_s = wpool.tile([P, NB, D], bf16, tag="wq")
            wk_s = wpool.tile([P, NB, D], bf16, tag="wk")
            wv_s = wpool.tile([P, NB, D], bf16, tag="wv")
            wo_s = wpool.tile([P, NB, D], bf16, tag="wo")
            w1_s = wpool.tile([P, NB, DFF], bf16, tag="w1")
            w2_s = wpool.tile([P, FB, D], bf16, tag="w2")
            wqs_s = wpool.tile([P, NB, H], f32, tag="wqs")
            bq_s = wpool.tile([P, NB], f32, tag="bq")
            bk_s = wpool.tile([P, NB], f32, tag="bk")
            b1_s = wpool.tile([P, FB], f32, tag="b1")
            bv_s = wpool.tile([1, D], f32, tag="bv")
            bo_s = wpool.tile([1, D], f32, tag="bo")
            b2_s = wpool.tile([1, D], f32, tag="b2")
            bqs_s = wpool.tile([1, H], f32, tag="bqs")
            for t, dsrc in ((wq_s, wq[l]), (wk_s, wk[l]), (wv_s, wv[l]),
                            (wqs_s, wqs[l]), (bq_s, bq[l]), (bk_s, bk[l]),
                            (bqs_s, bqsr[l]), (bv_s, bvr[l]), (wo_s, wo[l]),
                            (bo_s, bor[l]), (w1_s, w1[l]), (b1_s, b1r[l]),
                            (w2_s, w2[l]), (b2_s, b2r[l])):
                nc.sync.dma_start(out=t[:], in_=dsrc.ap())

            # q/k in transposed layout [d_out, i], bf16; bias per-partition
            qT = act.tile([P, NB, L], bf16, tag="qT")
            kT = act.tile([P, NB, L], bf16, tag="kT")
            for (dst, w_s, b_s) in ((qT, wq_s, bq_s), (kT, wk_s, bk_s)):
                for db in range(NB):
                    ps = pmain.tile([P, L], f32, tag="ps_main")
                    for ch in range(NB):
                        nc.tensor.matmul(ps, w_s[:, ch, db * P:(db + 1) * P],
                                         hTb[:, ch, :], start=(ch == 0), stop=(ch == NB - 1))
                    nc.vector.tensor_scalar_add(dst[:, db, :], ps, b_s[:, db:db + 1])

            # v in natural layout [i, d] extended with a ones column per head:
            # vx[:, ib, h*65 : h*65+64] = v_head, vx[:, ib, h*65+64] = 1.0
            # so the attention matmul's last output column is the softmax
            # denominator for that i-block.
            v_s = act.tile([P, NB, H * (E + 1)], bf16, tag="v", bufs=1)
            nc.vector.memset(v_s[:, :, E::E + 1], 1.0)
            for ib in range(NB):
                ps = pmain.tile([P, D], f32, tag="ps_main")
                for ch in range(NB):
                    nc.tensor.matmul(ps, hTb[:, ch, ib * P:(ib + 1) * P],
                                     wv_s[:, ch, :], start=(ch == 0), stop=False)
                nc.tensor.matmul(ps, ones[0:1, 0:P], bv_s[:], start=False, stop=True)
                for h in range(H):
                    nc.vector.tensor_copy(out=v_s[:, ib, h * (E + 1):h * (E + 1) + E],
                                          in_=ps[:, h * E:(h + 1) * E])

            # sigma projection in fp32: sig = h @ (Wq Ws) + (bq Ws + bs)
            sig_s = small.tile([P, NB, H], f32, tag="sig")
            for ib in range(NB):
                ps = psmall.tile([P, H], f32, tag="ps_small")
                for ch in range(NB):
                    nc.tensor.matmul(ps, hT[:, ch, ib * P:(ib + 1) * P],
                                     wqs_s[:, ch, :], start=(ch == 0), stop=False)
                nc.tensor.matmul(ps, ones[0:1, 0:P], bqs_s[:], start=False, stop=True)
                nc.vector.tensor_copy(out=sig_s[:, ib, :], in_=ps)

            # sigma transform chain (fp32, exp/ln only):
            #   sgm = 1/(1+exp(-5 sig)); sigma = 3^(sgm+1e-5) - 1
            #   s = -1/(2 sigma^2); lnc = -ln(sqrt(2pi) sigma)
            t_e = small.tile([P, NB, H], f32, tag="sg_t1")
            nc.scalar.activation(t_e[:], sig_s[:], AF.Exp, scale=-5.0)
            nc.vector.tensor_scalar_add(t_e[:], t_e[:], 1.0)
            sgm = small.tile([P, NB, H], f32, tag="sg_t2")
            nc.vector.reciprocal(sgm[:], t_e[:])
            sigma_f = small.tile([P, NB, H], f32, tag="sigma")
            nc.scalar.activation(sigma_f[:], sgm[:], AF.Exp, scale=LN3, bias=bias_u[:])
            nc.vector.tensor_scalar_add(sigma_f[:], sigma_f[:], -1.0)
            sigma_b = small.tile([P, NB, H], bf16, tag="sigma_b")
            nc.vector.tensor_copy(out=sigma_b[:], in_=sigma_f[:])
            sq = small.tile([P, NB, H], f32, tag="sg_t3")
            nc.vector.tensor_mul(out=sq[:], in0=sigma_f[:], in1=sigma_f[:])
            s_sb = small.tile([P, NB, H], f32, tag="s_exp")
            nc.vector.reciprocal(s_sb[:], sq[:])
            nc.vector.tensor_scalar_mul(s_sb[:], s_sb[:], -0.5)
            lnc = small.tile([P, NB, H], f32, tag="lnc")
            nc.scalar.activation(lnc[:], sigma_f[:], AF.Ln, scale=math.sqrt(2 * math.pi))
            nc.vector.tensor_scalar_mul(lnc[:], lnc[:], -1.0)

            # ---- Phase A: attention critical path (head pairs packed into
            # row groups 0-63 / 64-127 -> concurrent K=64 matmuls) ----------
            attn = act.tile([P, NB, D], bf16, tag="attn", bufs=1)
            nlz_all = small.tile([P, NB, H], f32, tag="nlz")
            for g in range(HG):
                expSTs = (head.tile([P, NB, L], bf16, tag="expST0", name="expST0"),
                          head.tile([P, NB, L], bf16, tag="expST1", name="expST1"))
                for jb in range(NB):
                    pss = []
                    for sub in range(2):
                        hp0 = E * sub
                        ps = pmain.tile([P, L], f32, tag="ps_main")
                        pss.append(ps)
                        nc.tensor.matmul(ps, kT[hp0:hp0 + E, g, jb * P:(jb + 1) * P],
                                         qT[hp0:hp0 + E, g, :], start=True, stop=True)
                    for sub in range(2):
                        nc.scalar.activation(expSTs[sub][:, jb, :], pss[sub],
                                             AF.Exp, scale=SCALE)
                for sub in range(2):
                    h = 2 * g + sub
                    expST = expSTs[sub]
                    rz_col = head.tile([P, NB], f32, tag="rz_col")
                    for ib in range(NB):
                        psA = psmall.tile([P, E + 1], f32, tag="ps_small")
                        for jb in range(NB):
                            nc.tensor.matmul(psA, expST[:, jb, ib * P:(ib + 1) * P],
                                             v_s[:, jb, h * (E + 1):(h + 1) * (E + 1)],
                                             start=(jb == 0), stop=(jb == NB - 1))
                        nc.vector.reciprocal(rz_col[:, ib:ib + 1], psA[:, E:E + 1])
                        nc.vector.tensor_scalar_mul(attn[:, ib, h * E:(h + 1) * E],
                                                    psA[:, 0:E], rz_col[:, ib:ib + 1])
                    nc.scalar.activation(nlz_all[:, :, h], rz_col[:], AF.Ln)

            # attn^T via PE transpose (bf16 128x128 tiles)
            attnT = act.tile([P, NB, L], bf16, tag="attnT", bufs=1)
            for db in range(NB):
                for ib in range(NB):
                    pst = ptrans.tile([P, P], bf16, tag="ps_tr")
                    nc.tensor.transpose(pst, attn[:, ib, db * P:(db + 1) * P], ident)
                    nc.vector.tensor_copy(out=attnT[:, db, ib * P:(ib + 1) * P], in_=pst)

            # h += attn @ Wo + bo   (in transposed layout)
            for db in range(NB):
                ps = pmain.tile([P, L], f32, tag="ps_main")
                for ch in range(NB):
                    nc.tensor.matmul(ps, wo_s[:, ch, db * P:(db + 1) * P],
                                     attnT[:, ch, :], start=(ch == 0), stop=False)
                nc.tensor.matmul(ps, bo_s[0:1, db * P:(db + 1) * P], ones[:],
                                 start=False, stop=True)
                nc.vector.tensor_add(out=hT[:, db, :], in0=hT[:, db, :], in1=ps)
                nc.vector.tensor_copy(out=hTb[:, db, :], in_=hT[:, db, :])

            # FFN
            y1 = act.tile([P, FB, L], bf16, tag="y1", bufs=1)
            for fb in range(FB):
                ps = pmain.tile([P, L], f32, tag="ps_main")
                for ch in range(NB):
                    nc.tensor.matmul(ps, w1_s[:, ch, fb * P:(fb + 1) * P],
                                     hTb[:, ch, :], start=(ch == 0), stop=(ch == NB - 1))
                nc.scalar.activation(y1[:, fb, :], ps, AF.Gelu, bias=b1_s[:, fb:fb + 1])
            for db in range(NB):
                ps = pmain.tile([P, L], f32, tag="ps_main")
                for fc in range(FB):
                    nc.tensor.matmul(ps, w2_s[:, fc, db * P:(db + 1) * P],
                                     y1[:, fc, :], start=(fc == 0), stop=False)
                nc.tensor.matmul(ps, b2_s[0:1, db * P:(db + 1) * P], ones[:],
                                 start=False, stop=True)
                nc.vector.tensor_add(out=hT[:, db, :], in0=hT[:, db, :], in1=ps)
                nc.vector.tensor_copy(out=hTb[:, db, :], in_=hT[:, db, :])

            # ---- Phase B (deferred off the residual critical path): series,
            # prior, sigma outputs; overlaps the FFN / next-layer QKV matmuls.
            for g in range(HG):
                sers = (head.tile([P, NB, L], bf16, tag="ser0", name="ser0"),
                        head.tile([P, NB, L], bf16, tag="ser1", name="ser1"))
                for ib in range(NB):
                    pss = []
                    for sub in range(2):
                        hp0 = E * sub
                        ps = pmain.tile([P, L], f32, tag="ps_main")
                        pss.append(ps)
                        nc.tensor.matmul(ps, qT[hp0:hp0 + E, g, ib * P:(ib + 1) * P],
                                         kT[hp0:hp0 + E, g, :], start=True, stop=True)
                    for sub in range(2):
                        h = 2 * g + sub
                        nc.scalar.activation(sers[sub][:, ib, :], pss[sub], AF.Exp,
                                             scale=SCALE,
                                             bias=nlz_all[:, ib, h:h + 1])
                for sub in range(2):
                    h = 2 * g + sub
                    nc.sync.dma_start(out=ser_d.ap()[l, h], in_=sers[sub][:])

                    # prior = exp(d2*s + lnc) on a +-1 block band; zeros outside
                    pri = head.tile([P, NB, L], bf16, tag="prior")
                    for ib in range(NB):
                        lo = max(0, ib - 1) * P
                        hi = min(NB - 1, ib + 1) * P + P
                        if lo > 0:
                            nc.gpsimd.memset(pri[:, ib, 0:lo], 0.0)
                        if hi < L:
                            nc.gpsimd.memset(pri[:, ib, hi:L], 0.0)
                        nc.scalar.activation(pri[:, ib, lo:hi], d2_sb[:, ib, lo:hi],
                                             AF.Exp, scale=s_sb[:, ib, h:h + 1],
                                             bias=lnc[:, ib, h:h + 1])
                    nc.sync.dma_start(out=pri_d.ap()[l, h], in_=pri[:])

                    # sigma broadcast to [*, L] (DVE stride-0 read)
                    sgb = head.tile([P, NB, L], bf16, tag="sigbc")
                    for ib in range(NB):
                        nc.vector.tensor_copy(
                            out=sgb[:, ib, :],
                            in_=sigma_b[:, ib, h:h + 1].to_broadcast((P, L)))
                    nc.sync.dma_start(out=sig_d.ap()[l, h], in_=sgb[:])

        # ---------------- output projection -------------------------------
        wp_s = persist.tile([P, NB, C_OUT], bf16)
        bp_s = persist.tile([1, C_OUT], f32)
        nc.sync.dma_start(out=wp_s[:], in_=wp.ap())
        nc.sync.dma_start(out=bp_s[:], in_=bpr.ap())
        out_sb = persist.tile([P, NB, C_OUT], f32)
        for ib in range(NB):
            ps = psmall.tile([P, C_OUT], f32, tag="ps_small")
            for ch in range(NB):
                nc.tensor.matmul(ps, hTb[:, ch, ib * P:(ib + 1) * P],
                                 wp_s[:, ch, :], start=(ch == 0), stop=False)
            nc.tensor.matmul(ps, ones[0:1, 0:P], bp_s[:], start=False, stop=True)
            nc.vector.tensor_copy(out=out_sb[:, ib, :], in_=ps)
        nc.sync.dma_start(out=out_d.ap(), in_=out_sb[:])

    # The act-table chooser maps each ACT function to the first table set
    # containing it, which puts exp and ln in different sets -> a ~2.7us
    # table reload per head. Restrict exp/ln membership to the shared
    # natural_log_exp_and_others set (dict order preserved, so set ids stay
    # valid) so the whole kernel runs on one resident set + gelu.
    import concourse.bacc as bacc_mod
    from concourse.hw_specs import get_activation_tables as _real_gat

    def _patched_gat(arch):
        tables = _real_gat(arch)
        exp_t = mybir.ActivationFunctionType.Exp
        ln_t = mybir.ActivationFunctionType.Ln
        for name, fns in tables.items():
            if name != "natural_log_exp_and_others":
                fns.discard(exp_t)
                fns.discard(ln_t)
        return tables

    bacc_mod.get_activation_tables = _patched_gat
    try:
        nc.compile()
    finally:
        bacc_mod.get_activation_tables = _real_gat
    return nc


def _prep_host_inputs(inputs):
    """Build the per-core in_maps (host-side layout transforms, bf16 casts)."""
    x = np.asarray(inputs["x"], np.float32)
    conv_w = np.asarray(inputs["conv_w"], np.float32)

    # positional embedding [L, D] -> transposed device layout [P, NB, L]
    pos_i = np.arange(L, dtype=np.float32)[:, None]
    div = np.exp(np.arange(0, D, 2, dtype=np.float32) * (-math.log(10000.0) / D))
    pe = np.zeros((L, D), np.float32)
    pe[:, 0::2] = np.sin(pos_i * div)
    pe[:, 1::2] = np.cos(pos_i * div)
    posT = np.ascontiguousarray(
        pe.T.reshape(NB, P, L).transpose(1, 0, 2))  # [P, NB, L]

    dist = np.abs(np.arange(L, dtype=np.float32)[:, None]
                  - np.arange(L, dtype=np.float32)[None, :])
    d2 = np.ascontiguousarray((dist ** 2).reshape(NB, P, L).transpose(1, 0, 2))

    wconv = np.transpose(conv_w, (2, 1, 0)).reshape(3 * C_IN, D)  # [(k c), o]

    def t_layout(w):  # [D, N] -> [P, D//P, N]
        return np.ascontiguousarray(
            w.reshape(w.shape[0] // P, P, w.shape[1]).transpose(1, 0, 2))

    def col_layout(b):  # [D] -> [P, D//P]
        return np.ascontiguousarray(b.reshape(b.shape[0] // P, P).T)

    shared = {
        "wconv0": wconv[:P].astype(BF16),
        "wconv1": wconv[P:].astype(BF16),
        "posT": posT,
        "d2": d2,
        "wp": t_layout(np.asarray(inputs["Wp"], np.float32)).astype(BF16),
        "bp": np.asarray(inputs["bp"], np.float32).reshape(1, C_OUT),
    }
    for l in range(NL):
        Wq = np.asarray(inputs["Wq"][l], np.float64)
        Ws = np.asarray(inputs["Ws"][l], np.float64)
        bq_ = np.asarray(inputs["bq"][l], np.float64)
        bs_ = np.asarray(inputs["bs"][l], np.float64)
        shared[f"wq{l}"] = t_layout(np.asarray(inputs["Wq"][l], np.float32)).astype(BF16)
        shared[f"wk{l}"] = t_layout(np.asarray(inputs["Wk"][l], np.float32)).astype(BF16)
        shared[f"wv{l}"] = t_layout(np.asarray(inputs["Wv"][l], np.float32)).astype(BF16)
        shared[f"wo{l}"] = t_layout(np.asarray(inputs["Wo"][l], np.float32)).astype(BF16)
        shared[f"w1{l}"] = t_layout(np.asarray(inputs["W1"][l], np.float32)).astype(BF16)
        shared[f"w2{l}"] = t_layout(np.asarray(inputs["W2"][l], np.float32)).astype(BF16)
        shared[f"wqs{l}"] = t_layout((Wq @ Ws).astype(np.float32))
        shared[f"bqs{l}"] = (bq_ @ Ws + bs_).astype(np.float32).reshape(1, H)
        shared[f"bq{l}"] = col_layout(np.asarray(inputs["bq"][l], np.float32))
        shared[f"bk{l}"] = col_layout(np.asarray(inputs["bk"][l], np.float32))
        shared[f"b1{l}"] = col_layout(np.asarray(inputs["b1"][l], np.float32))
        shared[f"bv{l}"] = np.asarray(inputs["bv"][l], np.float32).reshape(1, D)
        shared[f"bo{l}"] = np.asarray(inputs["bo"][l], np.float32).reshape(1, D)
        shared[f"b2{l}"] = np.asarray(inputs["b2"][l], np.float32).reshape(1, D)

    in_maps = []
    for b in range(B):
        xb = x[b]  # [L, C_IN]
        # xwin[(k c), i] = x[(i - 1 + k) % L, c]
        xwin = np.concatenate([np.roll(xb, 1 - k, axis=0).T for k in range(3)],
                              axis=0).astype(BF16)  # [165, L]
        m = dict(shared)
        m["xwin0"] = np.ascontiguousarray(xwin[:P])
        m["xwin1"] = np.ascontiguousarray(xwin[P:])
        in_maps.append(m)
    return in_maps


def run_cores(in_maps, trace=False, trace_kwargs=None):
    from concourse.bass_utils import run_bass_kernel_spmd

    if "nc" not in _GRAPH_CACHE:
        _GRAPH_CACHE["nc"] = build_graph()
    nc = _GRAPH_CACHE["nc"]
    return run_bass_kernel_spmd(nc, in_maps, core_ids=list(range(B)),
                                trace=trace, **(trace_kwargs or {}))


def _assemble(results):
    outs, sers, pris, sigs = [], [], [], []
    for b in range(B):
        r = results[b]
        o = np.asarray(r["out"], np.float32)          # [P, NB, C_OUT]
        outs.append(o.transpose(1, 0, 2).reshape(L, C_OUT))

        def big(name):
            a = np.asarray(r[name], np.float32)       # [NL, H, P, NB, L]
            return a.transpose(0, 1, 3, 2, 4).reshape(NL, H, L, L)

        sers.append(big("series"))
        pris.append(big("prior"))
        sigs.append(big("sigma"))
    return (np.stack(outs).astype(np.float32),
            np.stack(sers, axis=1).astype(np.float32),
            np.stack(pris, axis=1).astype(np.float32),
            np.stack(sigs, axis=1).astype(np.float32))


def kernel(**inputs):
    in_maps = _prep_host_inputs(inputs)
    res = run_cores(in_maps)
    return _assemble(res.results)
